# revision 19
# baseline (speedup 1.0000x reference)
"""GroupOfGESNCell Trainium2 kernel (fp16 pipeline).

Math (reference): 5 fixed-point iterations over G=4 groups:
    wiu = einsum('nf,ghf->gnh', X, W_ih)                     # [G,N,H]
    hx  <- tanh(wiu + L @ (hx @ W_hh_g^T))   per group       # N=8192, H=64
    out = concat_g(hx_g) -> [N, G*H=256]
The convergence early-exit never triggers for this regime; 5
unconditional iterations are exact. Iteration 0 starts from hx=0, so
hx1 = tanh(wiu): only 4 big matmuls of L are needed.

All operands are fp16 (X, Wih, L, Whh, hx, lin): fp16 matmuls run at
bf16 speed on TRN2 and cut the numerical error ~8x vs the bf16
version (sim: 1.6e-3 vs 1.2e-2 max rel err), leaving margin.

wiu is NOT precomputed/stored: each big accumulation chain starts
with one extra matmul (Wih^T slice x X^T chunk) that deposits wiu
into PSUM before the 64 L-tile matmuls accumulate on top. The evac
is then a single activation (tanh) reading PSUM once -- no DVE
tensor_add, no wiu SBUF buffer, less PSUM port traffic under the
next chain.

Distribution: row-shard L over 8 cores (1024 rows each), resident in
SBUF fp16 for all iterations. Per iteration each core computes
lin = hx @ Whh^T for its rows, AllGathers lin (fp16, split in two
gh-halves to pipeline against the PE), then runs the big chains.

Iteration 1 is collective-free: every core redundantly computes
lin_1 for ALL nodes from the full X (fp16), hiding the collective
warmup and the L-shard DMA under compute.

On-device layout: hx kept transposed (hxT [GH, n]) so every matmul
consumes natural layouts with zero on-device transposes.
"""

import sys

import numpy as np

sys.path.insert(0, "/opt/trn_rl_repo")

N, F, H, G = 8192, 128, 64, 4
GH = G * H  # 256
NCORES = 8
ROWS = N // NCORES  # 1024 rows of L / nodes per core
KT = N // 128  # 64 contraction tiles for the big matmul
JT = ROWS // 128  # 8 n-tiles per core
NITER = 5

_CACHE = {}


def _build_kernel():
    import concourse.mybir as mybir
    import concourse.tile as tile
    from concourse import bacc

    f16 = mybir.dt.float16
    f32 = mybir.dt.float32
    Tanh = mybir.ActivationFunctionType.Tanh

    nc = bacc.Bacc(
        "TRN2", target_bir_lowering=False, debug=False, num_devices=NCORES
    )

    # Per-core inputs (host-prepped, all fp16):
    #  LT    [N, ROWS] : L[rows_c, :].T  (contraction dim first)
    #  XT    [F, ROWS] : X[rows_c, :].T  (wiu lead-in matmuls)
    #  XTF   [F, N]    : full X.T        (iteration-1 redundant path)
    #  WihT  [F, GH]
    #  Wbd0/1 [128,128]: blockdiag(Whh_{2h}^T, Whh_{2h+1}^T)
    # Output: hxT_out [GH, ROWS] fp16 (host transposes + upcasts)
    lt_d = nc.declare_dram_parameter("LT", [N, ROWS], f16, isOutput=False)
    xt_d = nc.declare_dram_parameter("XT", [F, ROWS], f16, isOutput=False)
    xtf_d = nc.declare_dram_parameter("XTF", [F, N], f16, isOutput=False)
    wih_d = nc.declare_dram_parameter("WihT", [F, GH], f16, isOutput=False)
    wbd_d = [
        nc.declare_dram_parameter(f"Wbd{h}", [128, 128], f16, isOutput=False)
        for h in range(2)
    ]
    out_d = nc.declare_dram_parameter("hxT_out", [GH, ROWS], f16, isOutput=True)

    # Collective bounce buffers for iterations 2..4, per gh-half.
    cc_in = {
        (t, h): nc.dram_tensor(f"ccin_{t}_{h}", [ROWS, 128], f16)
        for t in range(2, NITER)
        for h in range(2)
    }
    cc_out = {
        (t, h): nc.dram_tensor(f"ccout_{t}_{h}", [N, 128], f16, addr_space="Shared")
        for t in range(2, NITER)
        for h in range(2)
    }
    warm_in = nc.dram_tensor("warm_in", [128, 16], f16)
    warm_out = nc.dram_tensor(
        "warm_out", [128 * NCORES, 16], f16, addr_space="Shared"
    )
    groups = [list(range(NCORES))]

    with tile.TileContext(nc) as tc:
        with (
            tc.tile_pool(name="lt", bufs=1) as lt_pool,
            tc.tile_pool(name="linf", bufs=1) as linf_pool,
            tc.tile_pool(name="hxt", bufs=2) as hxt_pool,
            tc.tile_pool(name="consts", bufs=1) as const_pool,
            tc.tile_pool(name="xtf", bufs=4) as xtf_pool,
            tc.tile_pool(name="hx1f", bufs=2) as hx1f_pool,
            tc.tile_pool(name="stage", bufs=1) as stage_pool,
            tc.tile_pool(name="outs", bufs=1) as out_pool,
            tc.tile_pool(name="bigp", bufs=3, space="PSUM") as bigp_pool,
            tc.tile_pool(name="smallp", bufs=2, space="PSUM") as smallp_pool,
        ):
            # dummy AllGather FIRST: the CC mesh/ncfw warm-up takes ~55us
            # after the trigger reaches the CC core, and the first real
            # AllGather queues behind it -- every ns earlier here moves the
            # iter1->iter2 boundary.
            nc.gpsimd.collective_compute(
                "AllGather",
                mybir.AluOpType.bypass,
                replica_groups=groups,
                ins=[warm_in[:, :]],
                outs=[warm_out[:, :]],
            )
            # ---- HAM prewarm ----
            # keep the PE busy 10.8->~15us so the HAM clock-gate reaches 8/8
            # before the first real matmul.
            junk_sb = const_pool.tile([128, 512], f16, tag="junk", name="junk")
            nc.gpsimd.memset(junk_sb[:], 0.0)
            for _ in range(8):
                ps = smallp_pool.tile(
                    [128, 512], f32, tag="hx1p", name="warmps"
                )
                nc.tensor.matmul(
                    ps[:], lhsT=junk_sb[:, 0:128], rhs=junk_sb[:], start=True, stop=True
                )

            # ---- constants: vector/scalar DMA queues so the sync queue
            # starts streaming the 16.8MB L-shard at t=0 (its tail used to
            # pace the iteration-1 chains by ~3.5us)
            xt_sb = const_pool.tile([F, ROWS], f16, tag="xt")
            wih_sb = const_pool.tile([F, GH], f16, tag="wih")
            nc.scalar.dma_start(wih_sb[:], wih_d[:, :])
            wbd_sb = [
                const_pool.tile([128, 128], f16, tag=f"wbd{h}", name=f"wbd{h}")
                for h in range(2)
            ]
            xtf_tiles = []
            for cc in range(4):
                t_ = xtf_pool.tile([128, 2048], f16, tag="xtfc", name="xtfc")
                xtf_tiles.append(t_)
            nc.scalar.dma_start(xtf_tiles[0][:], xtf_d[:, 0:2048])
            for h in range(2):
                nc.gpsimd.dma_start(wbd_sb[h][:], wbd_d[h][:, :])
            nc.gpsimd.dma_start(xtf_tiles[1][:], xtf_d[:, 2048:4096])
            nc.gpsimd.dma_start(xtf_tiles[2][:], xtf_d[:, 4096:6144])
            nc.gpsimd.dma_start(xtf_tiles[3][:], xtf_d[:, 6144:8192])
            nc.gpsimd.dma_start(xt_sb[:, 0:512], xt_d[:, 0:512])
            nc.gpsimd.dma_start(xt_sb[:, 512:ROWS], xt_d[:, 512:ROWS])

            # L-shard resident in SBUF: 16 tiles of [128, 4, ROWS] fp16
            lt_view = lt_d.rearrange("(i k p) n -> p (i k) n", p=128, k=4)
            lt_sb = []
            for i in range(16):
                t_ = lt_pool.tile([128, 4, ROWS], f16, tag=f"lt{i}", name=f"lt{i}")
                nc.sync.dma_start(t_[:], lt_view[:, 4 * i : 4 * i + 4, :])
                lt_sb.append(t_)

            def lt_slice(k, nh):
                return lt_sb[k // 4][:, k % 4, 512 * nh : 512 * nh + 512]

            # gathered lin, fp16: 8 tiles of [128, 8, 128] per gh-half
            linf = [
                [
                    linf_pool.tile(
                        [128, 8, 128], f16, tag=f"linf{h}_{i}", name=f"linf{h}_{i}"
                    )
                    for i in range(8)
                ]
                for h in range(2)
            ]

            # ---- iteration-1 lin for ALL nodes, computed locally ----
            # hx1 = tanh(X @ Wih^T); lin1 = hx1 @ blockdiag -> linf direct.
            # Each 512-node sub-chunk: one 512-wide matmul + tanh, then 4
            # small matmuls into a [128,4,128] psum view evacuated by ONE
            # copy (alternating DVE/Pool so neither queue backs up).
            for cc in range(4):  # [128, 2048] chunks of full X.T
                xtfc = xtf_tiles[cc]
                for m in range(2):
                    for s in range(4):
                        c = 4 * cc + s  # global 512-chunk 0..15
                        ps = smallp_pool.tile(
                            [128, 512], f32, tag="hx1p", name="h1ps"
                        )
                        nc.tensor.matmul(
                            ps[:],
                            lhsT=wih_sb[:, 128 * m : 128 * m + 128],
                            rhs=xtfc[:, 512 * s : 512 * s + 512],
                            start=True,
                            stop=True,
                        )
                        hx1c = hx1f_pool.tile([128, 512], f16, tag="hx1c", name="hx1c")
                        nc.scalar.activation(hx1c[:], ps[:], Tanh)
                        ps2 = smallp_pool.tile(
                            [128, 4, 128], f32, tag="sp", bufs=3, name="l1ps"
                        )
                        for j in range(4):
                            nc.tensor.matmul(
                                ps2[:, j, :],
                                lhsT=hx1c[:, 128 * j : 128 * j + 128],
                                rhs=wbd_sb[m][:],
                                start=True,
                                stop=True,
                            )
                        dst = linf[m][c // 2][:, 4 * (c % 2) : 4 * (c % 2) + 4, :]
                        nc.vector.tensor_copy(dst, ps2[:])

            def small_mm_and_ag(t, h, hx_tile):
                """lin cols [128h:128h+128] for local rows from hx_tile,
                then AllGather into linf[h]. 8 small matmuls land in one
                [128,8,128] psum view; two half-copies so the first DMA
                chunk leaves early."""
                stg = stage_pool.tile(
                    [128, JT, 128], f16, tag=f"stg{h}", name=f"stg{h}"
                )
                civ = cc_in[(t, h)].rearrange("(j p) c -> p j c", p=128)
                for half in range(2):
                    j0, j1 = JT // 2 * half, JT // 2 * (half + 1)
                    psl = smallp_pool.tile(
                        [128, 4, 128], f32, tag="sp", bufs=3, name="spl"
                    )
                    for j in range(j0, j1):
                        nc.tensor.matmul(
                            psl[:, j - j0, :],
                            lhsT=hx_tile[:, 128 * j : 128 * j + 128],
                            rhs=wbd_sb[h][:],
                            start=True,
                            stop=True,
                        )
                    nc.vector.tensor_copy(stg[:, j0:j1, :], psl[:])
                    nc.gpsimd.dma_start(civ[:, j0:j1, :], stg[:, j0:j1, :])
                nc.gpsimd.collective_compute(
                    "AllGather",
                    mybir.AluOpType.bypass,
                    replica_groups=groups,
                    ins=[cc_in[(t, h)][:, :]],
                    outs=[cc_out[(t, h)][:, :]],
                )
                cov = cc_out[(t, h)].rearrange("(i k p) c -> p (i k) c", p=128, k=8)
                # first k-tiles split out so the consuming matmul can start
                # earlier than a monolithic 8-k-tile load allows
                nc.sync.dma_start(linf[h][0][:, 0:2, :], cov[:, 0:2, :])
                nc.sync.dma_start(linf[h][0][:, 2:8, :], cov[:, 2:8, :])
                for i in range(1, 8):
                    nc.sync.dma_start(linf[h][i][:], cov[:, 8 * i : 8 * i + 8, :])

            def wiu_mm(ps, m, nh):
                # deposits the wiu term for gh-half m, node chunk nh into
                # psum as the opening matmul of the accumulation chain
                nc.tensor.matmul(
                    ps[:],
                    lhsT=wih_sb[:, 128 * m : 128 * m + 128],
                    rhs=xt_sb[:, 512 * nh : 512 * nh + 512],
                    start=True,
                    stop=False,
                )

            def big_mm(t, m, dst_tiles, last=False):
                """hxT_new[gh-half m] = tanh(psum) where psum accumulates
                wiu lead-in + 64 L-tile matmuls."""
                for nh in range(2):
                    ps = bigp_pool.tile([128, 512], f32, tag="big", name="bigps")
                    wiu_mm(ps, m, nh)
                    sl = slice(512 * nh, 512 * nh + 512)
                    for k in range(KT):
                        nc.tensor.matmul(
                            ps[:],
                            lhsT=linf[m][k // 8][:, k % 8, :],
                            rhs=lt_slice(k, nh),
                            start=False,
                            stop=(k == KT - 1),
                        )
                    if last:
                        # 256-col tanh/store pieces shorten the final drain
                        for q in range(2):
                            sp = slice(256 * q, 256 * q + 256)
                            so = slice(512 * nh + 256 * q, 512 * nh + 256 * q + 256)
                            oc = out_pool.tile(
                                [128, 256], f16, tag=f"oc{m}", bufs=2, name="oc"
                            )
                            nc.scalar.activation(oc[:], ps[:, sp], Tanh)
                            nc.sync.dma_start(
                                out_d[128 * m : 128 * m + 128, so], oc[:]
                            )
                    else:
                        nc.scalar.activation(dst_tiles[m][:, sl], ps[:], Tanh)

            def evac(ps_pair, m, dst):
                # 256-col pieces: the first small-mm (needs dst cols 0:128)
                # starts earlier
                for nh in range(2):
                    for q in range(2):
                        sp = slice(256 * q, 256 * q + 256)
                        sl = slice(512 * nh + 256 * q, 512 * nh + 256 * q + 256)
                        nc.scalar.activation(dst[:, sl], ps_pair[nh][:, sp], Tanh)

            def iter1_big(nxt):
                """Iteration-1 big matmuls, k-outer so the PE consumes each
                L tile as its DMA lands; m0 complete before m1 (sim-time
                ordering hint) so AG(2,0) hides under the m1 tail."""
                ps1 = {}

                def alloc_m(m):
                    for nh in range(2):
                        ps1[(m, nh)] = bigp_pool.tile(
                            [128, 512], f32, tag="big", name="bigps"
                        )

                def kgroups(m, i):
                    if i == 0:
                        for nh in range(2):
                            wiu_mm(ps1[(m, nh)], m, nh)
                    for k in range(4 * i, 4 * i + 4):
                        for nh in range(2):
                            nc.tensor.matmul(
                                ps1[(m, nh)][:],
                                lhsT=linf[m][k // 8][:, k % 8, :],
                                rhs=lt_slice(k, nh),
                                start=False,
                                stop=(k == KT - 1),
                            )

                with tc.tile_wait_until(0.03):
                    alloc_m(0)
                    for i in range(16):
                        kgroups(0, i)
                evac([ps1[(0, 0)], ps1[(0, 1)]], 0, nxt[0])
                small_mm_and_ag(2, 0, nxt[0])
                # sim-time hint: keep every m1 matmul behind all of m0 in
                # the static order (scheduling only, not a runtime wait)
                with tc.tile_wait_until(0.1):
                    alloc_m(1)
                    for i in range(16):
                        kgroups(1, i)
                evac([ps1[(1, 0)], ps1[(1, 1)]], 1, nxt[1])
                small_mm_and_ag(2, 1, nxt[1])

            # ---- software-pipelined iterations 1..4 ----
            # PE order: M0(t) | smallA(t+1)+AG_A | M1(t) | smallB(t+1)+AG_B
            hxt = None
            for t in range(1, NITER):
                last = t == NITER - 1
                if last:
                    nxt = [None, None]
                else:
                    nxt = [
                        hxt_pool.tile([128, ROWS], f16, tag="hxt", name="hxt")
                        for _ in range(2)
                    ]
                if t == 1:
                    iter1_big(nxt)
                else:
                    big_mm(t, 0, nxt, last=last)
                    if not last:
                        small_mm_and_ag(t + 1, 0, nxt[0])
                    big_mm(t, 1, nxt, last=last)
                    if not last:
                        small_mm_and_ag(t + 1, 1, nxt[1])
                hxt = nxt

    nc.compile()
    return nc


def _prep_inputs(X, L, W_ih, W_hh):
    h = np.float16
    Lh = np.ascontiguousarray(L.T).astype(h)  # [N, N] transposed, fp16
    XT = np.ascontiguousarray(X.T).astype(h)  # [F, N]
    WihT = np.ascontiguousarray(W_ih.reshape(GH, F).T).astype(h)  # [F, GH]
    wbd = [np.zeros((128, 128), np.float32) for _ in range(2)]
    for g in range(G):
        hh = g // 2
        o = (g % 2) * H
        wbd[hh][o : o + H, o : o + H] = W_hh[g].T
    in_maps = []
    for c in range(NCORES):
        sl = slice(c * ROWS, (c + 1) * ROWS)
        in_maps.append(
            {
                "LT": np.ascontiguousarray(Lh[:, sl]),
                "XT": np.ascontiguousarray(XT[:, sl]),
                "XTF": XT,
                "WihT": WihT,
                "Wbd0": wbd[0].astype(h),
                "Wbd1": wbd[1].astype(h),
            }
        )
    return in_maps


def kernel(X, L, W_ih, W_hh, trace=False):
    from concourse.bass_utils import run_bass_kernel_spmd

    X = np.asarray(X, np.float32)
    L = np.asarray(L, np.float32)
    W_ih = np.asarray(W_ih, np.float32)
    W_hh = np.asarray(W_hh, np.float32)

    if "nc" not in _CACHE:
        _CACHE["nc"] = _build_kernel()
    in_maps = _prep_inputs(X, L, W_ih, W_hh)
    res = run_bass_kernel_spmd(
        _CACHE["nc"], in_maps, list(range(NCORES)), trace=trace
    )
    out = np.empty((N, GH), np.float32)
    for c in range(NCORES):
        out[c * ROWS : (c + 1) * ROWS, :] = (
            res.results[c]["hxT_out"].astype(np.float32).T
        )
    _CACHE["last_result"] = res
    return out
